# revision 44
# baseline (speedup 1.0000x reference)
"""Trainium2 Bass kernel for nn_PredicateTensorModel.

Math (reference):
  subj/verb/obj[c,d] = weighted embedding bags (N=8 ids per batch row)
  A[c,p,q]  = sum_i w[i,p,q] verb[c,i]
  US[c,p,q] = sum_j u[j,p,q] subj[c,j]
  out[c,q]  = sum_p US[c,p,q] * A[c,p,q] * obj[c,p]

Sharding: tensor-parallel over trailing q axis (32 q's per core, 8 cores).

v3 design ([p,c] orientation):
  - Embedding rows are fetched with 12 dma_gather ops (1024 rows each --
    the SWDGE ring holds 1024 descriptors) from a host-compacted table
    (unique used ids remapped to int16 range), replacing the baseline's
    96 x ~1us single-row SWDGE gathers: ~16us Pool gen instead of ~100us.
  - Per q: psU'[p, c] = u_slice^T @ embT_s (lhsT = u[i, p-half], rhs =
    embT_s[i, c]); psA'[p, c] likewise; both [128, 1024] PSUM tiles
    (ph0|ph1 columns).  Act extracts psU'->USs (bf16), DVE computes
    H' = psA' * USs (one PSUM operand) and H'' = H' * objT (2x mode,
    all-SBUF bf16).  objT[p, c] is the o-bag in its natural [d, c]
    orientation - no transposes anywhere.
  - The p-reduction runs on the PE: out[q-slot, c] = onehot_q^T @ H''
    accumulated over 4 q x 2 ph into a [4, 512] PSUM tile, DMA'd
    directly to DRAM (host transposes [32, 512] -> [512, 32]).
  - Engine budget per core: PE ~73us (54.6 main + 13.7 reduce + bags),
    DVE ~63, Act ~47, Pool ~9, DMA ~43.
"""

import os
import sys

sys.path.insert(0, "/opt/trn_rl_repo")

import numpy as np
import ml_dtypes

N_CORES = 8
VOCAB, D, B, N = 50000, 256, 512, 8
QS = D // N_CORES  # 32 q columns per core
NCHUNK = B // 16  # 32 gather chunks of 16 batch rows
NQUART = 4
CHQ = NCHUNK // NQUART  # 8 chunks per quarter
NG = 3 * B * N  # total gathered rows (12288); compacted emb table size

bf16 = ml_dtypes.bfloat16

_PROG_CACHE = {}


def _build_program():
    import concourse.bass as bass
    import concourse.tile as tile
    import concourse.mybir as mybir
    from concourse import bacc
    from contextlib import ExitStack

    dt = mybir.dt
    # 2048-descriptor SWDGE ring: consecutive 1024-row dma_gathers
    # double-buffer through it instead of serializing gen->DMA->gen.
    nc = bacc.Bacc(dynamic_dma_scratch_size=32768)

    # compacted embedding table: rows = unique used ids (host-remapped)
    emb_p = nc.declare_dram_parameter("emb_g", [NG, D], dt.bfloat16, isOutput=False)
    w_p = nc.declare_dram_parameter("w_k", [D, QS, D], dt.bfloat16, isOutput=False)
    u_p = nc.declare_dram_parameter("u_k", [D, QS, D], dt.bfloat16, isOutput=False)
    # int16 dma_gather indices, 16-partition-wrapped per gather slice:
    # cols = [s ch0-8 (64) | s ch8-32 (192) | v0 (64) | v1 (192) | o (256)]
    ids_p = nc.declare_dram_parameter("idsg", [128, 768], dt.int16, isOutput=False)
    # S: columns = [S_s | S_v | S_o], each [128, B]
    S_p = nc.declare_dram_parameter("S", [128, 3 * B], dt.bfloat16, isOutput=False)
    # one-hot blocks for the PE reduce: block j (cols 4j..4j+3) = e_j outer ones
    oh_p = nc.declare_dram_parameter("oh", [128, 16], dt.bfloat16, isOutput=False)
    # output transposed: [q, c] per core; host transposes back
    out_p = nc.declare_dram_parameter("outT", [QS, B], dt.float32, isOutput=True)

    from concourse.library_config import mlp

    with ExitStack() as ctx:
        tc = ctx.enter_context(tile.TileContext(nc))
        nc.gpsimd.load_library(mlp)
        const_pool = ctx.enter_context(tc.tile_pool(name="const", bufs=1))
        gpool = ctx.enter_context(tc.tile_pool(name="gather", bufs=1))
        wu_pool = ctx.enter_context(tc.tile_pool(name="wu", bufs=1))
        embT_pool = ctx.enter_context(tc.tile_pool(name="embT", bufs=1))
        uss_pool = ctx.enter_context(tc.tile_pool(name="uss", bufs=4))
        h_pool = ctx.enter_context(tc.tile_pool(name="h", bufs=10))
        h2_pool = ctx.enter_context(tc.tile_pool(name="h2", bufs=5))
        outs_pool = ctx.enter_context(tc.tile_pool(name="outs", bufs=2))
        pu_pool = ctx.enter_context(tc.tile_pool(name="pu", bufs=2, space="PSUM"))
        pa_pool = ctx.enter_context(tc.tile_pool(name="pa", bufs=2, space="PSUM"))

        # ---- constants ----
        ids = const_pool.tile([128, 768], dt.int16, name="ids", tag="ids")
        nc.sync.dma_start(out=ids[:], in_=ids_p[:])
        S = const_pool.tile([128, 3 * B], dt.bfloat16, name="S", tag="S")
        nc.sync.dma_start(out=S[:], in_=S_p[:])
        OH = const_pool.tile([128, 16], dt.bfloat16, name="oh", tag="oh")
        nc.sync.dma_start(out=OH[:], in_=oh_p[:])

        # ---- resident w/u tiles: [128 i(half), 32q x 256p] per ic ----
        w_res = []
        u_res = []
        for ic in range(2):
            w_res.append(wu_pool.tile([128, QS * D], dt.bfloat16, name=f"w{ic}", tag=f"w{ic}"))
            u_res.append(wu_pool.tile([128, QS * D], dt.bfloat16, name=f"u{ic}", tag=f"u{ic}"))

        def load_piece(res, par, p):
            # one [128, 2048] piece = 8 q columns for one ic half
            for ic in range(2):
                nc.sync.dma_start(
                    out=res[ic][:, p * 2048 : (p + 1) * 2048],
                    in_=par[ic * 128 : (ic + 1) * 128, p * 8 : (p + 1) * 8, :],
                )

        # ---- gather tiles: one [128, 8*D] tile per (type, quarter) ----
        # each dma_gather moves 1024 rows (8 chunks) = 1024 descriptors,
        # fitting the 1024-descriptor SWDGE ring carveout.
        V = {
            t: [
                gpool.tile([128, 8 * D], dt.bfloat16, name=f"V{t}{k}", tag=f"V{t}{k}")
                for k in range(NQUART)
            ]
            for t in "svo"
        }
        # idsg column order: [s q0..q3 | v q0..q3 | o q0..q3], 64 cols each
        TOFF = {"s": 0, "v": 256, "o": 512}

        def gather(t, k):
            off = TOFF[t] + k * 64
            nc.gpsimd.dma_gather(
                out_ap=V[t][k][:].rearrange("p (k c) -> p k c", c=D),
                in_ap=emb_p[:],
                idxs_ap=ids[:, off : off + 64],
                num_idxs=1024,
                num_idxs_reg=1024,
                elem_size=D,
            )

        def vslice(t, ck):
            return V[t][ck // 8][:, (ck % 8) * D : (ck % 8 + 1) * D]

        # ---- bags: embT[t][dh] = [128 d-half, 512 c] bf16 ----
        embT = {
            t: [
                embT_pool.tile([128, B], dt.bfloat16, name=f"eT{t}{dh}", tag=f"eT{t}{dh}")
                for dh in range(2)
            ]
            for t in "sv"
        }
        # objT: [128 p(-half along cols), ph*512 + c] matching H' layout
        objT = embT_pool.tile([128, 2 * B], dt.bfloat16, name="objT", tag="objT")

        def bags(t, k):
            toff = {"s": 0, "v": B, "o": 2 * B}[t]
            psE = pa_pool.tile([128, 256], dt.float32, name=f"psE{t}{k}", tag="pa")
            for dh in range(2):
                for c8 in range(CHQ):
                    ck = k * CHQ + c8
                    nc.tensor.matmul(
                        out=psE[:, dh * 128 + c8 * 16 : dh * 128 + (c8 + 1) * 16],
                        lhsT=vslice(t, ck)[:, dh * 128 : (dh + 1) * 128],
                        rhs=S[:, toff + ck * 16 : toff + (ck + 1) * 16],
                        start=True,
                        stop=True,
                    )
            cb = k * 128
            for dh in range(2):
                if t == "o":
                    nc.scalar.copy(
                        out=objT[:, dh * 512 + cb : dh * 512 + cb + 128],
                        in_=psE[:, dh * 128 : (dh + 1) * 128],
                    )
                else:
                    nc.scalar.copy(
                        out=embT[t][dh][:, cb : cb + 128],
                        in_=psE[:, dh * 128 : (dh + 1) * 128],
                    )

        # ---- per-q pipeline ----
        H2s = {}

        def q_tile(q):
            # psU'[p, c]: cols = ph*512 + c
            psU = pu_pool.tile([128, 1024], dt.float32, name="psU", tag="pu")
            for ph in range(2):
                for ic in range(2):
                    nc.tensor.matmul(
                        out=psU[:, ph * 512 : (ph + 1) * 512],
                        lhsT=u_res[ic][:, q * 256 + ph * 128 : q * 256 + (ph + 1) * 128],
                        rhs=embT["s"][ic][:],
                        start=(ic == 0),
                        stop=(ic == 1),
                    )
            USs = uss_pool.tile([128, 1024], dt.bfloat16, name="USs", tag="USs")
            nc.scalar.copy(out=USs[:], in_=psU[:])
            psA = pa_pool.tile([128, 1024], dt.float32, name="psA", tag="pa")
            for ph in range(2):
                for ic in range(2):
                    nc.tensor.matmul(
                        out=psA[:, ph * 512 : (ph + 1) * 512],
                        lhsT=w_res[ic][:, q * 256 + ph * 128 : q * 256 + (ph + 1) * 128],
                        rhs=embT["v"][ic][:],
                        start=(ic == 0),
                        stop=(ic == 1),
                    )
            H = h_pool.tile([128, 1024], dt.bfloat16, name="H", tag="H")
            nc.vector.tensor_mul(H[:], psA[:], USs[:])
            H2s[q] = H

        def reduce_group(g):
            # H2 = H * objT (deferred here so it follows the o-bags), then
            # psR[0:4, c] = sum_{ph} onehot_qq^T @ H2(q=4g+qq)[ph] for qq in 0..3
            psR = pu_pool.tile([128, 512], dt.float32, name="psR", tag="pu")
            for qq in range(4):
                H = H2s.pop(4 * g + qq)
                H2 = h2_pool.tile([128, 1024], dt.bfloat16, name="H2", tag="H2")
                nc.vector.tensor_mul(H2[:], H[:], objT[:])
                for ph in range(2):
                    nc.tensor.matmul(
                        out=psR[0:4, :],
                        lhsT=OH[:, qq * 4 : (qq + 1) * 4],
                        rhs=H2[:, ph * 512 : (ph + 1) * 512],
                        start=(qq == 0 and ph == 0),
                        stop=(qq == 3 and ph == 1),
                    )
            outS = outs_pool.tile([4, B], dt.float32, name=f"oS{g}", tag="oS")
            nc.scalar.copy(out=outS[:], in_=psR[0:4, :])
            nc.sync.dma_start(out=out_p[4 * g : 4 * g + 4, :], in_=outS[:])

        # ================= program order =================
        gather("s", 0)
        gather("v", 0)
        gather("s", 1)
        gather("v", 1)
        load_piece(u_res, u_p, 0)
        gather("s", 2)
        gather("v", 2)
        load_piece(w_res, w_p, 0)
        gather("s", 3)
        gather("v", 3)
        gather("o", 0)
        gather("o", 1)
        gather("o", 2)
        gather("o", 3)
        load_piece(u_res, u_p, 1)
        load_piece(w_res, w_p, 1)
        load_piece(u_res, u_p, 2)
        load_piece(w_res, w_p, 2)
        load_piece(u_res, u_p, 3)
        load_piece(w_res, w_p, 3)

        for k in range(NQUART):
            bags("s", k)
            bags("v", k)
        for q in range(4):
            q_tile(q)
        for k in range(NQUART):
            bags("o", k)
        # reduce_group(g) is emitted ~3 q_tiles after its last H so the
        # in-order PE queue never waits on DVE's H2 chain.
        for q in range(4, QS):
            q_tile(q)
            if q == 5:
                reduce_group(0)
            elif q >= 10 and (q - 6) % 4 == 0:
                reduce_group((q - 6) // 4)
        reduce_group(7)

    nc.finalize()
    return nc


def _get_program():
    if "nc" not in _PROG_CACHE:
        _PROG_CACHE["nc"] = _build_program()
    return _PROG_CACHE["nc"]


def _host_prep(inputs):
    """Shard + lay out inputs for the 8 cores. Returns list of in_maps."""
    ids = {}
    wts = {}
    for t, idk, wk in (
        ("s", "subj_id", "subj_w"),
        ("v", "verb_id", "verb_w"),
        ("o", "obj_id", "obj_w"),
    ):
        ids[t] = np.asarray(inputs[idk]).astype(np.int32)
        wts[t] = np.asarray(inputs[wk]).astype(np.float32)

    emb = np.asarray(inputs["emb"], dtype=np.float32)
    w = np.asarray(inputs["w"], dtype=np.float32)
    u = np.asarray(inputs["u"], dtype=np.float32)

    emb_b = emb.astype(bf16)
    # [i, p, q] -> [i, q, p], contiguous, then shard q
    wT = np.ascontiguousarray(w.transpose(0, 2, 1)).astype(bf16)
    uT = np.ascontiguousarray(u.transpose(0, 2, 1)).astype(bf16)

    ids_r = {}
    S_cols = []
    for t in "svo":
        # partition p = (c % 16)*8 + n ; column = chunk ck = c // 16
        ids_r[t] = ids[t].reshape(NCHUNK, 16, 8).transpose(1, 2, 0).reshape(128, NCHUNK)
        Sm = np.zeros((16, 8, NCHUNK, 16), np.float32)
        wr = wts[t].reshape(NCHUNK, 16, 8).transpose(1, 2, 0)
        j = np.arange(16)
        Sm[j[:, None, None], np.arange(8)[None, :, None], np.arange(NCHUNK)[None, None, :], j[:, None, None]] = wr
        S_cols.append(Sm.reshape(128, B))
    S_all = np.ascontiguousarray(np.concatenate(S_cols, axis=1)).astype(bf16)

    # Compact the vocab to the unique used ids so indices fit int16 for
    # dma_gather; remap slice indices into the compacted table.
    uniq = np.unique(np.concatenate([ids_r[t].ravel() for t in "svo"]))
    emb_g = np.zeros((NG, D), dtype=bf16)
    emb_g[: len(uniq)] = emb_b[uniq]

    def slice_cols(t, c0, c1):
        # gathered row t_loc = (ck-c0)*128 + ppos -> idx tile [16, (c1-c0)*8]
        arr = ids_r[t][:, c0:c1]  # [128 ppos, nck]
        arr2 = arr.reshape(8, 16, c1 - c0).transpose(1, 2, 0).reshape(16, (c1 - c0) * 8)
        return np.searchsorted(uniq, arr2).astype(np.int16)

    idsg16 = np.concatenate(
        [slice_cols(t, k * 8, (k + 1) * 8) for t in "svo" for k in range(NQUART)],
        axis=1,
    )  # [16, 768]
    idsg = np.ascontiguousarray(np.tile(idsg16, (8, 1)))  # [128, 768]

    # one-hot blocks: OH[:, 4j:4j+4] has column j all ones, others zero
    oh = np.zeros((128, 16), np.float32)
    for jj in range(4):
        oh[:, 4 * jj + jj] = 1.0
    oh = oh.astype(bf16)

    in_maps = []
    for k in range(N_CORES):
        m = {
            "emb_g": emb_g,
            "w_k": np.ascontiguousarray(wT[:, k * QS : (k + 1) * QS, :]),
            "u_k": np.ascontiguousarray(uT[:, k * QS : (k + 1) * QS, :]),
            "idsg": idsg,
            "S": S_all,
            "oh": oh,
        }
        in_maps.append(m)
    return in_maps


def kernel(**inputs) -> np.ndarray:
    from concourse.bass_utils import run_bass_kernel_spmd

    nc = _get_program()
    in_maps = _host_prep(inputs)
    trace = bool(int(os.environ.get("KTRACE", "0")))
    res = run_bass_kernel_spmd(
        nc, in_maps, core_ids=list(range(N_CORES)), trace=trace
    )
    if trace:
        _PROG_CACHE["last_result"] = res
    # per-core outT is [QS, B]; transpose and concat along q
    out = np.concatenate(
        [res.results[k]["outT"].astype(np.float32).T for k in range(N_CORES)], axis=1
    )
    return out


# revision 46
# speedup vs baseline: 1.0060x; 1.0060x over previous
"""Trainium2 Bass kernel for nn_PredicateTensorModel.

Math (reference):
  subj/verb/obj[c,d] = weighted embedding bags (N=8 ids per batch row)
  A[c,p,q]  = sum_i w[i,p,q] verb[c,i]
  US[c,p,q] = sum_j u[j,p,q] subj[c,j]
  out[c,q]  = sum_p US[c,p,q] * A[c,p,q] * obj[c,p]

Sharding: tensor-parallel over trailing q axis (32 q's per core, 8 cores).

v3 design ([p,c] orientation):
  - Embedding rows are fetched with 12 dma_gather ops (1024 rows each --
    the SWDGE ring holds 1024 descriptors) from a host-compacted table
    (unique used ids remapped to int16 range), replacing the baseline's
    96 x ~1us single-row SWDGE gathers: ~16us Pool gen instead of ~100us.
  - Per q: psU'[p, c] = u_slice^T @ embT_s (lhsT = u[i, p-half], rhs =
    embT_s[i, c]); psA'[p, c] likewise; both [128, 1024] PSUM tiles
    (ph0|ph1 columns).  Act extracts psU'->USs (bf16), DVE computes
    H' = psA' * USs (one PSUM operand) and H'' = H' * objT (2x mode,
    all-SBUF bf16).  objT[p, c] is the o-bag in its natural [d, c]
    orientation - no transposes anywhere.
  - The p-reduction runs on the PE: out[q-slot, c] = onehot_q^T @ H''
    accumulated over 4 q x 2 ph into a [4, 512] PSUM tile, DMA'd
    directly to DRAM (host transposes [32, 512] -> [512, 32]).
  - Engine budget per core: PE ~73us (54.6 main + 13.7 reduce + bags),
    DVE ~63, Act ~47, Pool ~9, DMA ~43.
"""

import os
import sys

sys.path.insert(0, "/opt/trn_rl_repo")

import numpy as np
import ml_dtypes

N_CORES = 8
VOCAB, D, B, N = 50000, 256, 512, 8
QS = D // N_CORES  # 32 q columns per core
NCHUNK = B // 16  # 32 gather chunks of 16 batch rows
NQUART = 4
CHQ = NCHUNK // NQUART  # 8 chunks per quarter
NG = 3 * B * N  # total gathered rows (12288); compacted emb table size

bf16 = ml_dtypes.bfloat16

_PROG_CACHE = {}


def _build_program():
    import concourse.bass as bass
    import concourse.tile as tile
    import concourse.mybir as mybir
    from concourse import bacc
    from contextlib import ExitStack

    dt = mybir.dt
    # 2048-descriptor SWDGE ring: consecutive 1024-row dma_gathers
    # double-buffer through it instead of serializing gen->DMA->gen.
    nc = bacc.Bacc(dynamic_dma_scratch_size=32768)

    # compacted embedding table: rows = unique used ids (host-remapped)
    emb_p = nc.declare_dram_parameter("emb_g", [NG, D], dt.bfloat16, isOutput=False)
    w_p = nc.declare_dram_parameter("w_k", [D, QS, D], dt.bfloat16, isOutput=False)
    u_p = nc.declare_dram_parameter("u_k", [D, QS, D], dt.bfloat16, isOutput=False)
    # int16 dma_gather indices, 16-partition-wrapped per gather slice:
    # cols = [s ch0-8 (64) | s ch8-32 (192) | v0 (64) | v1 (192) | o (256)]
    ids_p = nc.declare_dram_parameter("idsg", [128, 768], dt.int16, isOutput=False)
    # S: columns = [S_s | S_v | S_o], each [128, B]
    S_p = nc.declare_dram_parameter("S", [128, 3 * B], dt.bfloat16, isOutput=False)
    # one-hot blocks for the PE reduce: block j (cols 4j..4j+3) = e_j outer ones
    oh_p = nc.declare_dram_parameter("oh", [128, 16], dt.bfloat16, isOutput=False)
    # output transposed: [q, c] per core; host transposes back
    out_p = nc.declare_dram_parameter("outT", [QS, B], dt.float32, isOutput=True)

    from concourse.library_config import mlp

    with ExitStack() as ctx:
        tc = ctx.enter_context(tile.TileContext(nc))
        nc.gpsimd.load_library(mlp)
        const_pool = ctx.enter_context(tc.tile_pool(name="const", bufs=1))
        gpool = ctx.enter_context(tc.tile_pool(name="gather", bufs=1))
        wu_pool = ctx.enter_context(tc.tile_pool(name="wu", bufs=1))
        embT_pool = ctx.enter_context(tc.tile_pool(name="embT", bufs=1))
        uss_pool = ctx.enter_context(tc.tile_pool(name="uss", bufs=4))
        h_pool = ctx.enter_context(tc.tile_pool(name="h", bufs=10))
        h2_pool = ctx.enter_context(tc.tile_pool(name="h2", bufs=5))
        outs_pool = ctx.enter_context(tc.tile_pool(name="outs", bufs=2))
        pu_pool = ctx.enter_context(tc.tile_pool(name="pu", bufs=2, space="PSUM"))
        pa_pool = ctx.enter_context(tc.tile_pool(name="pa", bufs=2, space="PSUM"))

        # ---- constants ----
        ids = const_pool.tile([128, 768], dt.int16, name="ids", tag="ids")
        nc.sync.dma_start(out=ids[:], in_=ids_p[:])
        S = const_pool.tile([128, 3 * B], dt.bfloat16, name="S", tag="S")
        nc.sync.dma_start(out=S[:], in_=S_p[:])
        OH = const_pool.tile([128, 16], dt.bfloat16, name="oh", tag="oh")
        nc.sync.dma_start(out=OH[:], in_=oh_p[:])

        # ---- resident w/u tiles: [128 i(half), 32q x 256p] per ic ----
        w_res = []
        u_res = []
        for ic in range(2):
            w_res.append(wu_pool.tile([128, QS * D], dt.bfloat16, name=f"w{ic}", tag=f"w{ic}"))
            u_res.append(wu_pool.tile([128, QS * D], dt.bfloat16, name=f"u{ic}", tag=f"u{ic}"))

        def load_piece(res, par, p):
            # one [128, 2048] piece = 8 q columns for one ic half
            for ic in range(2):
                nc.sync.dma_start(
                    out=res[ic][:, p * 2048 : (p + 1) * 2048],
                    in_=par[ic * 128 : (ic + 1) * 128, p * 8 : (p + 1) * 8, :],
                )

        # ---- gather tiles: one [128, 8*D] tile per (type, quarter) ----
        # each dma_gather moves 1024 rows (8 chunks) = 1024 descriptors,
        # fitting the 1024-descriptor SWDGE ring carveout.
        V = {
            t: [
                gpool.tile([128, 8 * D], dt.bfloat16, name=f"V{t}{k}", tag=f"V{t}{k}")
                for k in range(NQUART)
            ]
            for t in "svo"
        }
        # idsg column order: [s q0..q3 | v q0..q3 | o q0..q3], 64 cols each
        TOFF = {"s": 0, "v": 256, "o": 512}

        def gather(t, k):
            off = TOFF[t] + k * 64
            nc.gpsimd.dma_gather(
                out_ap=V[t][k][:].rearrange("p (k c) -> p k c", c=D),
                in_ap=emb_p[:],
                idxs_ap=ids[:, off : off + 64],
                num_idxs=1024,
                num_idxs_reg=1024,
                elem_size=D,
            )

        def vslice(t, ck):
            return V[t][ck // 8][:, (ck % 8) * D : (ck % 8 + 1) * D]

        # ---- bags: embT[t][dh] = [128 d-half, 512 c] bf16 ----
        embT = {
            t: [
                embT_pool.tile([128, B], dt.bfloat16, name=f"eT{t}{dh}", tag=f"eT{t}{dh}")
                for dh in range(2)
            ]
            for t in "sv"
        }
        # objT: [128 p(-half along cols), ph*512 + c] matching H' layout
        objT = embT_pool.tile([128, 2 * B], dt.bfloat16, name="objT", tag="objT")

        def bags(t, k):
            toff = {"s": 0, "v": B, "o": 2 * B}[t]
            psE = pa_pool.tile([128, 256], dt.float32, name=f"psE{t}{k}", tag="pa")
            for dh in range(2):
                for c8 in range(CHQ):
                    ck = k * CHQ + c8
                    nc.tensor.matmul(
                        out=psE[:, dh * 128 + c8 * 16 : dh * 128 + (c8 + 1) * 16],
                        lhsT=vslice(t, ck)[:, dh * 128 : (dh + 1) * 128],
                        rhs=S[:, toff + ck * 16 : toff + (ck + 1) * 16],
                        start=True,
                        stop=True,
                    )
            cb = k * 128
            for dh in range(2):
                if t == "o":
                    nc.scalar.copy(
                        out=objT[:, dh * 512 + cb : dh * 512 + cb + 128],
                        in_=psE[:, dh * 128 : (dh + 1) * 128],
                    )
                else:
                    nc.scalar.copy(
                        out=embT[t][dh][:, cb : cb + 128],
                        in_=psE[:, dh * 128 : (dh + 1) * 128],
                    )

        # ---- per-q pipeline ----
        H2s = {}

        def q_tile(q):
            # psU'[p, c]: cols = ph*512 + c
            psU = pu_pool.tile([128, 1024], dt.float32, name="psU", tag="pu")
            for ph in range(2):
                for ic in range(2):
                    nc.tensor.matmul(
                        out=psU[:, ph * 512 : (ph + 1) * 512],
                        lhsT=u_res[ic][:, q * 256 + ph * 128 : q * 256 + (ph + 1) * 128],
                        rhs=embT["s"][ic][:],
                        start=(ic == 0),
                        stop=(ic == 1),
                    )
            USs = uss_pool.tile([128, 1024], dt.bfloat16, name="USs", tag="USs")
            nc.scalar.copy(out=USs[:], in_=psU[:])
            psA = pa_pool.tile([128, 1024], dt.float32, name="psA", tag="pa")
            for ph in range(2):
                for ic in range(2):
                    nc.tensor.matmul(
                        out=psA[:, ph * 512 : (ph + 1) * 512],
                        lhsT=w_res[ic][:, q * 256 + ph * 128 : q * 256 + (ph + 1) * 128],
                        rhs=embT["v"][ic][:],
                        start=(ic == 0),
                        stop=(ic == 1),
                    )
            H = h_pool.tile([128, 1024], dt.bfloat16, name="H", tag="H")
            nc.vector.tensor_mul(H[:], psA[:], USs[:])
            H2s[q] = H

        def reduce_group(g):
            # H2 = H * objT (deferred here so it follows the o-bags), then
            # psR[0:4, c] = sum_{ph} onehot_qq^T @ H2(q=4g+qq)[ph] for qq in 0..3
            psR = pu_pool.tile([128, 512], dt.float32, name="psR", tag="pu")
            for qq in range(4):
                H = H2s.pop(4 * g + qq)
                H2 = h2_pool.tile([128, 1024], dt.bfloat16, name="H2", tag="H2")
                nc.vector.tensor_mul(H2[:], H[:], objT[:])
                for ph in range(2):
                    nc.tensor.matmul(
                        out=psR[0:4, :],
                        lhsT=OH[:, qq * 4 : (qq + 1) * 4],
                        rhs=H2[:, ph * 512 : (ph + 1) * 512],
                        start=(qq == 0 and ph == 0),
                        stop=(qq == 3 and ph == 1),
                    )
            outS = outs_pool.tile([4, B], dt.float32, name=f"oS{g}", tag="oS")
            nc.scalar.copy(out=outS[:], in_=psR[0:4, :])
            nc.sync.dma_start(out=out_p[4 * g : 4 * g + 4, :], in_=outS[:])

        # ================= program order =================
        gather("s", 0)
        gather("v", 0)
        gather("s", 1)
        gather("v", 1)
        gather("s", 2)
        gather("v", 2)
        gather("s", 3)
        gather("v", 3)
        load_piece(u_res, u_p, 0)
        gather("o", 0)
        load_piece(w_res, w_p, 0)
        gather("o", 1)
        gather("o", 2)
        gather("o", 3)
        load_piece(u_res, u_p, 1)
        load_piece(w_res, w_p, 1)
        load_piece(u_res, u_p, 2)
        load_piece(w_res, w_p, 2)
        load_piece(u_res, u_p, 3)
        load_piece(w_res, w_p, 3)

        for k in range(NQUART):
            bags("s", k)
            bags("v", k)
        for q in range(4):
            q_tile(q)
        for k in range(NQUART):
            bags("o", k)
        # reduce_group(g) is emitted ~3 q_tiles after its last H so the
        # in-order PE queue never waits on DVE's H2 chain.
        for q in range(4, QS):
            q_tile(q)
            if q == 6:
                reduce_group(0)
            elif q >= 10 and (q - 6) % 4 == 0:
                reduce_group((q - 6) // 4)
        reduce_group(7)

    nc.finalize()
    return nc


def _get_program():
    if "nc" not in _PROG_CACHE:
        _PROG_CACHE["nc"] = _build_program()
    return _PROG_CACHE["nc"]


def _host_prep(inputs):
    """Shard + lay out inputs for the 8 cores. Returns list of in_maps."""
    ids = {}
    wts = {}
    for t, idk, wk in (
        ("s", "subj_id", "subj_w"),
        ("v", "verb_id", "verb_w"),
        ("o", "obj_id", "obj_w"),
    ):
        ids[t] = np.asarray(inputs[idk]).astype(np.int32)
        wts[t] = np.asarray(inputs[wk]).astype(np.float32)

    emb = np.asarray(inputs["emb"], dtype=np.float32)
    w = np.asarray(inputs["w"], dtype=np.float32)
    u = np.asarray(inputs["u"], dtype=np.float32)

    emb_b = emb.astype(bf16)
    # [i, p, q] -> [i, q, p], contiguous, then shard q
    wT = np.ascontiguousarray(w.transpose(0, 2, 1)).astype(bf16)
    uT = np.ascontiguousarray(u.transpose(0, 2, 1)).astype(bf16)

    ids_r = {}
    S_cols = []
    for t in "svo":
        # partition p = (c % 16)*8 + n ; column = chunk ck = c // 16
        ids_r[t] = ids[t].reshape(NCHUNK, 16, 8).transpose(1, 2, 0).reshape(128, NCHUNK)
        Sm = np.zeros((16, 8, NCHUNK, 16), np.float32)
        wr = wts[t].reshape(NCHUNK, 16, 8).transpose(1, 2, 0)
        j = np.arange(16)
        Sm[j[:, None, None], np.arange(8)[None, :, None], np.arange(NCHUNK)[None, None, :], j[:, None, None]] = wr
        S_cols.append(Sm.reshape(128, B))
    S_all = np.ascontiguousarray(np.concatenate(S_cols, axis=1)).astype(bf16)

    # Compact the vocab to the unique used ids so indices fit int16 for
    # dma_gather; remap slice indices into the compacted table.
    uniq = np.unique(np.concatenate([ids_r[t].ravel() for t in "svo"]))
    emb_g = np.zeros((NG, D), dtype=bf16)
    emb_g[: len(uniq)] = emb_b[uniq]

    def slice_cols(t, c0, c1):
        # gathered row t_loc = (ck-c0)*128 + ppos -> idx tile [16, (c1-c0)*8]
        arr = ids_r[t][:, c0:c1]  # [128 ppos, nck]
        arr2 = arr.reshape(8, 16, c1 - c0).transpose(1, 2, 0).reshape(16, (c1 - c0) * 8)
        return np.searchsorted(uniq, arr2).astype(np.int16)

    idsg16 = np.concatenate(
        [slice_cols(t, k * 8, (k + 1) * 8) for t in "svo" for k in range(NQUART)],
        axis=1,
    )  # [16, 768]
    idsg = np.ascontiguousarray(np.tile(idsg16, (8, 1)))  # [128, 768]

    # one-hot blocks: OH[:, 4j:4j+4] has column j all ones, others zero
    oh = np.zeros((128, 16), np.float32)
    for jj in range(4):
        oh[:, 4 * jj + jj] = 1.0
    oh = oh.astype(bf16)

    in_maps = []
    for k in range(N_CORES):
        m = {
            "emb_g": emb_g,
            "w_k": np.ascontiguousarray(wT[:, k * QS : (k + 1) * QS, :]),
            "u_k": np.ascontiguousarray(uT[:, k * QS : (k + 1) * QS, :]),
            "idsg": idsg,
            "S": S_all,
            "oh": oh,
        }
        in_maps.append(m)
    return in_maps


def kernel(**inputs) -> np.ndarray:
    from concourse.bass_utils import run_bass_kernel_spmd

    nc = _get_program()
    in_maps = _host_prep(inputs)
    trace = bool(int(os.environ.get("KTRACE", "0")))
    res = run_bass_kernel_spmd(
        nc, in_maps, core_ids=list(range(N_CORES)), trace=trace
    )
    if trace:
        _PROG_CACHE["last_result"] = res
    # per-core outT is [QS, B]; transpose and concat along q
    out = np.concatenate(
        [res.results[k]["outT"].astype(np.float32).T for k in range(N_CORES)], axis=1
    )
    return out


# revision 49
# speedup vs baseline: 1.1416x; 1.1348x over previous
"""Trainium2 Bass kernel for nn_PredicateTensorModel.

Math (reference):
  subj/verb/obj[c,d] = weighted embedding bags (N=8 ids per batch row)
  A[c,p,q]  = sum_i w[i,p,q] verb[c,i]
  US[c,p,q] = sum_j u[j,p,q] subj[c,j]
  out[c,q]  = sum_p US[c,p,q] * A[c,p,q] * obj[c,p]

Sharding: tensor-parallel over trailing q axis (32 q's per core, 8 cores).

v3 design ([p,c] orientation):
  - Embedding rows are fetched with 12 dma_gather ops (1024 rows each --
    the SWDGE ring holds 1024 descriptors) from a host-compacted table
    (unique used ids remapped to int16 range), replacing the baseline's
    96 x ~1us single-row SWDGE gathers: ~16us Pool gen instead of ~100us.
  - Per q: psU'[p, c] = u_slice^T @ embT_s (lhsT = u[i, p-half], rhs =
    embT_s[i, c]); psA'[p, c] likewise; both [128, 1024] PSUM tiles
    (ph0|ph1 columns).  Act extracts psU'->USs (bf16), DVE computes
    H' = psA' * USs (one PSUM operand) and H'' = H' * objT (2x mode,
    all-SBUF bf16).  objT[p, c] is the o-bag in its natural [d, c]
    orientation - no transposes anywhere.
  - The p-reduction runs on the PE: out[q-slot, c] = onehot_q^T @ H''
    accumulated over 4 q x 2 ph into a [4, 512] PSUM tile, DMA'd
    directly to DRAM (host transposes [32, 512] -> [512, 32]).
  - Engine budget per core: PE ~73us (54.6 main + 13.7 reduce + bags),
    DVE ~63, Act ~47, Pool ~9, DMA ~43.
"""

import os
import sys

sys.path.insert(0, "/opt/trn_rl_repo")

import numpy as np
import ml_dtypes

N_CORES = 8
VOCAB, D, B, N = 50000, 256, 512, 8
QS = D // N_CORES  # 32 q columns per core
NCHUNK = B // 16  # 32 gather chunks of 16 batch rows
NQUART = 4
CHQ = NCHUNK // NQUART  # 8 chunks per quarter
NG = 3 * B * N  # total gathered rows (12288); compacted emb table size

bf16 = ml_dtypes.bfloat16

_PROG_CACHE = {}


def _build_program():
    import concourse.bass as bass
    import concourse.tile as tile
    import concourse.mybir as mybir
    from concourse import bacc
    from contextlib import ExitStack

    dt = mybir.dt
    # 2048-descriptor SWDGE ring: consecutive 1024-row dma_gathers
    # double-buffer through it instead of serializing gen->DMA->gen.
    nc = bacc.Bacc(dynamic_dma_scratch_size=32768)

    # compacted embedding table: rows = unique used ids (host-remapped)
    emb_p = nc.declare_dram_parameter("emb_g", [NG, D], dt.bfloat16, isOutput=False)
    w_p = nc.declare_dram_parameter("w_k", [D, QS, D], dt.bfloat16, isOutput=False)
    u_p = nc.declare_dram_parameter("u_k", [D, QS, D], dt.bfloat16, isOutput=False)
    # int16 dma_gather indices, 16-partition-wrapped per gather slice:
    # cols = [s ch0-8 (64) | s ch8-32 (192) | v0 (64) | v1 (192) | o (256)]
    ids_p = nc.declare_dram_parameter("idsg", [128, 768], dt.int16, isOutput=False)
    # S: columns = [S_s | S_v | S_o], each [128, B]
    S_p = nc.declare_dram_parameter("S", [128, 3 * B], dt.bfloat16, isOutput=False)
    # one-hot blocks for the PE reduce: block j (cols 4j..4j+3) = e_j outer ones
    oh_p = nc.declare_dram_parameter("oh", [128, 16], dt.bfloat16, isOutput=False)
    # output transposed: [q, c] per core; host transposes back
    out_p = nc.declare_dram_parameter("outT", [QS, B], dt.float32, isOutput=True)

    from concourse.library_config import mlp

    with ExitStack() as ctx:
        tc = ctx.enter_context(tile.TileContext(nc))
        nc.gpsimd.load_library(mlp)
        const_pool = ctx.enter_context(tc.tile_pool(name="const", bufs=1))
        gpool = ctx.enter_context(tc.tile_pool(name="gather", bufs=1))
        wu_pool = ctx.enter_context(tc.tile_pool(name="wu", bufs=1))
        embT_pool = ctx.enter_context(tc.tile_pool(name="embT", bufs=1))
        uss_pool = ctx.enter_context(tc.tile_pool(name="uss", bufs=4))
        h_pool = ctx.enter_context(tc.tile_pool(name="h", bufs=10))
        h2_pool = ctx.enter_context(tc.tile_pool(name="h2", bufs=5))
        outs_pool = ctx.enter_context(tc.tile_pool(name="outs", bufs=2))
        pu_pool = ctx.enter_context(tc.tile_pool(name="pu", bufs=2, space="PSUM"))
        pa_pool = ctx.enter_context(tc.tile_pool(name="pa", bufs=2, space="PSUM"))

        # ---- constants ----
        ids = const_pool.tile([128, 768], dt.int16, name="ids", tag="ids")
        nc.sync.dma_start(out=ids[:], in_=ids_p[:])
        S = const_pool.tile([128, 3 * B], dt.bfloat16, name="S", tag="S")
        nc.sync.dma_start(out=S[:], in_=S_p[:])
        OH = const_pool.tile([128, 16], dt.bfloat16, name="oh", tag="oh")
        nc.sync.dma_start(out=OH[:], in_=oh_p[:])

        # ---- resident w/u tiles: [128 i(half), 32q x 256p] per ic ----
        w_res = []
        u_res = []
        for ic in range(2):
            w_res.append(wu_pool.tile([128, QS * D], dt.bfloat16, name=f"w{ic}", tag=f"w{ic}"))
            u_res.append(wu_pool.tile([128, QS * D], dt.bfloat16, name=f"u{ic}", tag=f"u{ic}"))

        def load_piece(res, par, p):
            # one [128, 2048] piece = 8 q columns for one ic half.
            # SWDGE (Pool queue) so the DMA requests stay ordered *behind*
            # the gathers emitted before them in program order; HWDGE
            # requests would jump the queue and starve the gather stream.
            for ic in range(2):
                nc.gpsimd.dma_start(
                    out=res[ic][:, p * 2048 : (p + 1) * 2048],
                    in_=par[ic * 128 : (ic + 1) * 128, p * 8 : (p + 1) * 8, :],
                )

        # ---- gather tiles: one [128, 8*D] tile per (type, quarter) ----
        # each dma_gather moves 1024 rows (8 chunks) = 1024 descriptors,
        # fitting the 1024-descriptor SWDGE ring carveout.
        V = {
            t: [
                gpool.tile([128, 8 * D], dt.bfloat16, name=f"V{t}{k}", tag=f"V{t}{k}")
                for k in range(NQUART)
            ]
            for t in "svo"
        }
        # idsg column order: [s q0..q3 | v q0..q3 | o q0..q3], 64 cols each
        TOFF = {"s": 0, "v": 256, "o": 512}

        def gather(t, k):
            off = TOFF[t] + k * 64
            nc.gpsimd.dma_gather(
                out_ap=V[t][k][:].rearrange("p (k c) -> p k c", c=D),
                in_ap=emb_p[:],
                idxs_ap=ids[:, off : off + 64],
                num_idxs=1024,
                num_idxs_reg=1024,
                elem_size=D,
            )

        def vslice(t, ck):
            return V[t][ck // 8][:, (ck % 8) * D : (ck % 8 + 1) * D]

        # ---- bags: embT[t][dh] = [128 d-half, 512 c] bf16 ----
        embT = {
            t: [
                embT_pool.tile([128, B], dt.bfloat16, name=f"eT{t}{dh}", tag=f"eT{t}{dh}")
                for dh in range(2)
            ]
            for t in "sv"
        }
        # objT: [128 p(-half along cols), ph*512 + c] matching H' layout
        objT = embT_pool.tile([128, 2 * B], dt.bfloat16, name="objT", tag="objT")

        def bags(t, k):
            toff = {"s": 0, "v": B, "o": 2 * B}[t]
            psE = pa_pool.tile([128, 256], dt.float32, name=f"psE{t}{k}", tag="pa")
            for dh in range(2):
                for c8 in range(CHQ):
                    ck = k * CHQ + c8
                    nc.tensor.matmul(
                        out=psE[:, dh * 128 + c8 * 16 : dh * 128 + (c8 + 1) * 16],
                        lhsT=vslice(t, ck)[:, dh * 128 : (dh + 1) * 128],
                        rhs=S[:, toff + ck * 16 : toff + (ck + 1) * 16],
                        start=True,
                        stop=True,
                    )
            cb = k * 128
            for dh in range(2):
                if t == "o":
                    nc.scalar.copy(
                        out=objT[:, dh * 512 + cb : dh * 512 + cb + 128],
                        in_=psE[:, dh * 128 : (dh + 1) * 128],
                    )
                else:
                    nc.scalar.copy(
                        out=embT[t][dh][:, cb : cb + 128],
                        in_=psE[:, dh * 128 : (dh + 1) * 128],
                    )

        # ---- per-q pipeline ----
        H2s = {}

        def q_tile(q):
            # psU'[p, c]: cols = ph*512 + c
            psU = pu_pool.tile([128, 1024], dt.float32, name="psU", tag="pu")
            for ph in range(2):
                for ic in range(2):
                    nc.tensor.matmul(
                        out=psU[:, ph * 512 : (ph + 1) * 512],
                        lhsT=u_res[ic][:, q * 256 + ph * 128 : q * 256 + (ph + 1) * 128],
                        rhs=embT["s"][ic][:],
                        start=(ic == 0),
                        stop=(ic == 1),
                    )
            USs = uss_pool.tile([128, 1024], dt.bfloat16, name="USs", tag="USs")
            nc.scalar.copy(out=USs[:], in_=psU[:])
            psA = pa_pool.tile([128, 1024], dt.float32, name="psA", tag="pa")
            for ph in range(2):
                for ic in range(2):
                    nc.tensor.matmul(
                        out=psA[:, ph * 512 : (ph + 1) * 512],
                        lhsT=w_res[ic][:, q * 256 + ph * 128 : q * 256 + (ph + 1) * 128],
                        rhs=embT["v"][ic][:],
                        start=(ic == 0),
                        stop=(ic == 1),
                    )
            H = h_pool.tile([128, 1024], dt.bfloat16, name="H", tag="H")
            nc.vector.tensor_mul(H[:], psA[:], USs[:])
            H2s[q] = H

        def reduce_group(g):
            # H2 = H * objT (deferred here so it follows the o-bags), then
            # psR[0:4, c] = sum_{ph} onehot_qq^T @ H2(q=4g+qq)[ph] for qq in 0..3
            psR = pu_pool.tile([128, 512], dt.float32, name="psR", tag="pu")
            for qq in range(4):
                H = H2s.pop(4 * g + qq)
                H2 = h2_pool.tile([128, 1024], dt.bfloat16, name="H2", tag="H2")
                nc.vector.tensor_mul(H2[:], H[:], objT[:])
                for ph in range(2):
                    nc.tensor.matmul(
                        out=psR[0:4, :],
                        lhsT=OH[:, qq * 4 : (qq + 1) * 4],
                        rhs=H2[:, ph * 512 : (ph + 1) * 512],
                        start=(qq == 0 and ph == 0),
                        stop=(qq == 3 and ph == 1),
                    )
            outS = outs_pool.tile([4, B], dt.float32, name=f"oS{g}", tag="oS")
            nc.scalar.copy(out=outS[:], in_=psR[0:4, :])
            nc.sync.dma_start(out=out_p[4 * g : 4 * g + 4, :], in_=outS[:])

        # ================= program order =================
        gather("s", 0)
        gather("v", 0)
        gather("s", 1)
        gather("v", 1)
        gather("s", 2)
        gather("v", 2)
        gather("s", 3)
        gather("v", 3)
        load_piece(u_res, u_p, 0)
        load_piece(w_res, w_p, 0)
        gather("o", 0)
        gather("o", 1)
        gather("o", 2)
        gather("o", 3)
        load_piece(u_res, u_p, 1)
        load_piece(w_res, w_p, 1)
        load_piece(u_res, u_p, 2)
        load_piece(w_res, w_p, 2)
        load_piece(u_res, u_p, 3)
        load_piece(w_res, w_p, 3)

        for k in range(NQUART):
            bags("s", k)
            bags("v", k)
        for q in range(4):
            q_tile(q)
        # reduce_group(g) is emitted ~3 q_tiles after its last H so the
        # in-order PE queue never waits on DVE's H2 chain; o-bags are spread
        # through q4-q7 to track the o-gather arrivals.
        for q in range(4, QS):
            q_tile(q)
            if 4 <= q <= 7:
                bags("o", q - 4)
            if q == 8:
                reduce_group(0)
            elif q >= 10 and (q - 6) % 4 == 0:
                reduce_group((q - 6) // 4)
        reduce_group(7)

    nc.finalize()
    return nc


def _get_program():
    if "nc" not in _PROG_CACHE:
        _PROG_CACHE["nc"] = _build_program()
    return _PROG_CACHE["nc"]


def _host_prep(inputs):
    """Shard + lay out inputs for the 8 cores. Returns list of in_maps."""
    ids = {}
    wts = {}
    for t, idk, wk in (
        ("s", "subj_id", "subj_w"),
        ("v", "verb_id", "verb_w"),
        ("o", "obj_id", "obj_w"),
    ):
        ids[t] = np.asarray(inputs[idk]).astype(np.int32)
        wts[t] = np.asarray(inputs[wk]).astype(np.float32)

    emb = np.asarray(inputs["emb"], dtype=np.float32)
    w = np.asarray(inputs["w"], dtype=np.float32)
    u = np.asarray(inputs["u"], dtype=np.float32)

    emb_b = emb.astype(bf16)
    # [i, p, q] -> [i, q, p], contiguous, then shard q
    wT = np.ascontiguousarray(w.transpose(0, 2, 1)).astype(bf16)
    uT = np.ascontiguousarray(u.transpose(0, 2, 1)).astype(bf16)

    ids_r = {}
    S_cols = []
    for t in "svo":
        # partition p = (c % 16)*8 + n ; column = chunk ck = c // 16
        ids_r[t] = ids[t].reshape(NCHUNK, 16, 8).transpose(1, 2, 0).reshape(128, NCHUNK)
        Sm = np.zeros((16, 8, NCHUNK, 16), np.float32)
        wr = wts[t].reshape(NCHUNK, 16, 8).transpose(1, 2, 0)
        j = np.arange(16)
        Sm[j[:, None, None], np.arange(8)[None, :, None], np.arange(NCHUNK)[None, None, :], j[:, None, None]] = wr
        S_cols.append(Sm.reshape(128, B))
    S_all = np.ascontiguousarray(np.concatenate(S_cols, axis=1)).astype(bf16)

    # Compact the vocab to the unique used ids so indices fit int16 for
    # dma_gather; remap slice indices into the compacted table.
    uniq = np.unique(np.concatenate([ids_r[t].ravel() for t in "svo"]))
    emb_g = np.zeros((NG, D), dtype=bf16)
    emb_g[: len(uniq)] = emb_b[uniq]

    def slice_cols(t, c0, c1):
        # gathered row t_loc = (ck-c0)*128 + ppos -> idx tile [16, (c1-c0)*8]
        arr = ids_r[t][:, c0:c1]  # [128 ppos, nck]
        arr2 = arr.reshape(8, 16, c1 - c0).transpose(1, 2, 0).reshape(16, (c1 - c0) * 8)
        return np.searchsorted(uniq, arr2).astype(np.int16)

    idsg16 = np.concatenate(
        [slice_cols(t, k * 8, (k + 1) * 8) for t in "svo" for k in range(NQUART)],
        axis=1,
    )  # [16, 768]
    idsg = np.ascontiguousarray(np.tile(idsg16, (8, 1)))  # [128, 768]

    # one-hot blocks: OH[:, 4j:4j+4] has column j all ones, others zero
    oh = np.zeros((128, 16), np.float32)
    for jj in range(4):
        oh[:, 4 * jj + jj] = 1.0
    oh = oh.astype(bf16)

    in_maps = []
    for k in range(N_CORES):
        m = {
            "emb_g": emb_g,
            "w_k": np.ascontiguousarray(wT[:, k * QS : (k + 1) * QS, :]),
            "u_k": np.ascontiguousarray(uT[:, k * QS : (k + 1) * QS, :]),
            "idsg": idsg,
            "S": S_all,
            "oh": oh,
        }
        in_maps.append(m)
    return in_maps


def kernel(**inputs) -> np.ndarray:
    from concourse.bass_utils import run_bass_kernel_spmd

    nc = _get_program()
    in_maps = _host_prep(inputs)
    trace = bool(int(os.environ.get("KTRACE", "0")))
    res = run_bass_kernel_spmd(
        nc, in_maps, core_ids=list(range(N_CORES)), trace=trace
    )
    if trace:
        _PROG_CACHE["last_result"] = res
    # per-core outT is [QS, B]; transpose and concat along q
    out = np.concatenate(
        [res.results[k]["outT"].astype(np.float32).T for k in range(N_CORES)], axis=1
    )
    return out


# revision 50
# speedup vs baseline: 1.1608x; 1.0168x over previous
"""Trainium2 Bass kernel for nn_PredicateTensorModel.

Math (reference):
  subj/verb/obj[c,d] = weighted embedding bags (N=8 ids per batch row)
  A[c,p,q]  = sum_i w[i,p,q] verb[c,i]
  US[c,p,q] = sum_j u[j,p,q] subj[c,j]
  out[c,q]  = sum_p US[c,p,q] * A[c,p,q] * obj[c,p]

Sharding: tensor-parallel over trailing q axis (32 q's per core, 8 cores).

v3 design ([p,c] orientation):
  - Embedding rows are fetched with 12 dma_gather ops (1024 rows each --
    the SWDGE ring holds 1024 descriptors) from a host-compacted table
    (unique used ids remapped to int16 range), replacing the baseline's
    96 x ~1us single-row SWDGE gathers: ~16us Pool gen instead of ~100us.
  - Per q: psU'[p, c] = u_slice^T @ embT_s (lhsT = u[i, p-half], rhs =
    embT_s[i, c]); psA'[p, c] likewise; both [128, 1024] PSUM tiles
    (ph0|ph1 columns).  Act extracts psU'->USs (bf16), DVE computes
    H' = psA' * USs (one PSUM operand) and H'' = H' * objT (2x mode,
    all-SBUF bf16).  objT[p, c] is the o-bag in its natural [d, c]
    orientation - no transposes anywhere.
  - The p-reduction runs on the PE: out[q-slot, c] = onehot_q^T @ H''
    accumulated over 4 q x 2 ph into a [4, 512] PSUM tile, DMA'd
    directly to DRAM (host transposes [32, 512] -> [512, 32]).
  - Engine budget per core: PE ~73us (54.6 main + 13.7 reduce + bags),
    DVE ~63, Act ~47, Pool ~9, DMA ~43.
"""

import os
import sys

sys.path.insert(0, "/opt/trn_rl_repo")

import numpy as np
import ml_dtypes

N_CORES = 8
VOCAB, D, B, N = 50000, 256, 512, 8
QS = D // N_CORES  # 32 q columns per core
NCHUNK = B // 16  # 32 gather chunks of 16 batch rows
NQUART = 4
CHQ = NCHUNK // NQUART  # 8 chunks per quarter
NG = 3 * B * N  # total gathered rows (12288); compacted emb table size

bf16 = ml_dtypes.bfloat16

_PROG_CACHE = {}


def _build_program():
    import concourse.bass as bass
    import concourse.tile as tile
    import concourse.mybir as mybir
    from concourse import bacc
    from contextlib import ExitStack

    dt = mybir.dt
    # 2048-descriptor SWDGE ring: consecutive 1024-row dma_gathers
    # double-buffer through it instead of serializing gen->DMA->gen.
    nc = bacc.Bacc(dynamic_dma_scratch_size=32768)

    # compacted embedding table: rows = unique used ids (host-remapped)
    emb_p = nc.declare_dram_parameter("emb_g", [NG, D], dt.bfloat16, isOutput=False)
    w_p = nc.declare_dram_parameter("w_k", [D, QS, D], dt.bfloat16, isOutput=False)
    u_p = nc.declare_dram_parameter("u_k", [D, QS, D], dt.bfloat16, isOutput=False)
    # int16 dma_gather indices, 16-partition-wrapped per gather slice:
    # cols = [s ch0-8 (64) | s ch8-32 (192) | v0 (64) | v1 (192) | o (256)]
    ids_p = nc.declare_dram_parameter("idsg", [128, 768], dt.int16, isOutput=False)
    # S: columns = [S_s | S_v | S_o], each [128, B]
    S_p = nc.declare_dram_parameter("S", [128, 3 * B], dt.bfloat16, isOutput=False)
    # one-hot blocks for the PE reduce: block j (cols 4j..4j+3) = e_j outer ones
    oh_p = nc.declare_dram_parameter("oh", [128, 16], dt.bfloat16, isOutput=False)
    # output transposed: [q, c] per core; host transposes back
    out_p = nc.declare_dram_parameter("outT", [QS, B], dt.float32, isOutput=True)

    from concourse.library_config import mlp

    with ExitStack() as ctx:
        tc = ctx.enter_context(tile.TileContext(nc))
        nc.gpsimd.load_library(mlp)
        const_pool = ctx.enter_context(tc.tile_pool(name="const", bufs=1))
        gpool = ctx.enter_context(tc.tile_pool(name="gather", bufs=1))
        wu_pool = ctx.enter_context(tc.tile_pool(name="wu", bufs=1))
        embT_pool = ctx.enter_context(tc.tile_pool(name="embT", bufs=1))
        uss_pool = ctx.enter_context(tc.tile_pool(name="uss", bufs=4))
        h_pool = ctx.enter_context(tc.tile_pool(name="h", bufs=10))
        h2_pool = ctx.enter_context(tc.tile_pool(name="h2", bufs=5))
        outs_pool = ctx.enter_context(tc.tile_pool(name="outs", bufs=2))
        pu_pool = ctx.enter_context(tc.tile_pool(name="pu", bufs=2, space="PSUM"))
        pa_pool = ctx.enter_context(tc.tile_pool(name="pa", bufs=2, space="PSUM"))

        # ---- constants ----
        ids = const_pool.tile([128, 768], dt.int16, name="ids", tag="ids")
        nc.sync.dma_start(out=ids[:], in_=ids_p[:])
        S = const_pool.tile([128, 3 * B], dt.bfloat16, name="S", tag="S")
        nc.sync.dma_start(out=S[:], in_=S_p[:])
        OH = const_pool.tile([128, 16], dt.bfloat16, name="oh", tag="oh")
        nc.sync.dma_start(out=OH[:], in_=oh_p[:])

        # ---- resident w/u tiles: [128 i(half), 32q x 256p] per ic ----
        w_res = []
        u_res = []
        for ic in range(2):
            w_res.append(wu_pool.tile([128, QS * D], dt.bfloat16, name=f"w{ic}", tag=f"w{ic}"))
            u_res.append(wu_pool.tile([128, QS * D], dt.bfloat16, name=f"u{ic}", tag=f"u{ic}"))

        def load_piece(res, par, p):
            # one [128, 2048] piece = 8 q columns for one ic half.
            # SWDGE (Pool queue) so the DMA requests stay ordered *behind*
            # the gathers emitted before them in program order; HWDGE
            # requests would jump the queue and starve the gather stream.
            for ic in range(2):
                nc.gpsimd.dma_start(
                    out=res[ic][:, p * 2048 : (p + 1) * 2048],
                    in_=par[ic * 128 : (ic + 1) * 128, p * 8 : (p + 1) * 8, :],
                )

        # ---- gather tiles: one [128, 8*D] tile per (type, quarter) ----
        # each dma_gather moves 1024 rows (8 chunks) = 1024 descriptors,
        # fitting the 1024-descriptor SWDGE ring carveout.
        V = {
            t: [
                gpool.tile([128, 8 * D], dt.bfloat16, name=f"V{t}{k}", tag=f"V{t}{k}")
                for k in range(NQUART)
            ]
            for t in "svo"
        }
        # idsg column order: [s q0..q3 | v q0..q3 | o q0..q3], 64 cols each
        TOFF = {"s": 0, "v": 256, "o": 512}

        def gather(t, k):
            off = TOFF[t] + k * 64
            nc.gpsimd.dma_gather(
                out_ap=V[t][k][:].rearrange("p (k c) -> p k c", c=D),
                in_ap=emb_p[:],
                idxs_ap=ids[:, off : off + 64],
                num_idxs=1024,
                num_idxs_reg=1024,
                elem_size=D,
            )

        def vslice(t, ck):
            return V[t][ck // 8][:, (ck % 8) * D : (ck % 8 + 1) * D]

        # ---- bags: embT[t][dh] = [128 d-half, 512 c] bf16 ----
        embT = {
            t: [
                embT_pool.tile([128, B], dt.bfloat16, name=f"eT{t}{dh}", tag=f"eT{t}{dh}")
                for dh in range(2)
            ]
            for t in "sv"
        }
        # objT: [128 p(-half along cols), ph*512 + c] matching H' layout
        objT = embT_pool.tile([128, 2 * B], dt.bfloat16, name="objT", tag="objT")

        def bags(t, k):
            toff = {"s": 0, "v": B, "o": 2 * B}[t]
            psE = pu_pool.tile([128, 256], dt.float32, name=f"psE{t}{k}", tag="pu")
            for dh in range(2):
                for c8 in range(CHQ):
                    ck = k * CHQ + c8
                    nc.tensor.matmul(
                        out=psE[:, dh * 128 + c8 * 16 : dh * 128 + (c8 + 1) * 16],
                        lhsT=vslice(t, ck)[:, dh * 128 : (dh + 1) * 128],
                        rhs=S[:, toff + ck * 16 : toff + (ck + 1) * 16],
                        start=True,
                        stop=True,
                    )
            cb = k * 128
            for dh in range(2):
                if t == "o":
                    nc.scalar.copy(
                        out=objT[:, dh * 512 + cb : dh * 512 + cb + 128],
                        in_=psE[:, dh * 128 : (dh + 1) * 128],
                    )
                else:
                    nc.scalar.copy(
                        out=embT[t][dh][:, cb : cb + 128],
                        in_=psE[:, dh * 128 : (dh + 1) * 128],
                    )

        # ---- per-q pipeline ----
        H2s = {}

        def q_tile(q):
            # psU'[p, c]: cols = ph*512 + c
            psU = pu_pool.tile([128, 1024], dt.float32, name="psU", tag="pu")
            for ph in range(2):
                for ic in range(2):
                    nc.tensor.matmul(
                        out=psU[:, ph * 512 : (ph + 1) * 512],
                        lhsT=u_res[ic][:, q * 256 + ph * 128 : q * 256 + (ph + 1) * 128],
                        rhs=embT["s"][ic][:],
                        start=(ic == 0),
                        stop=(ic == 1),
                    )
            USs = uss_pool.tile([128, 1024], dt.bfloat16, name="USs", tag="USs")
            nc.scalar.copy(out=USs[:], in_=psU[:])
            psA = pa_pool.tile([128, 1024], dt.float32, name="psA", tag="pa")
            for ph in range(2):
                for ic in range(2):
                    nc.tensor.matmul(
                        out=psA[:, ph * 512 : (ph + 1) * 512],
                        lhsT=w_res[ic][:, q * 256 + ph * 128 : q * 256 + (ph + 1) * 128],
                        rhs=embT["v"][ic][:],
                        start=(ic == 0),
                        stop=(ic == 1),
                    )
            H = h_pool.tile([128, 1024], dt.bfloat16, name="H", tag="H")
            nc.vector.tensor_mul(H[:], psA[:], USs[:])
            H2s[q] = H

        def reduce_group(g):
            # H2 = H * objT (deferred here so it follows the o-bags), then
            # psR[0:4, c] = sum_{ph} onehot_qq^T @ H2(q=4g+qq)[ph] for qq in 0..3
            psR = pu_pool.tile([128, 512], dt.float32, name="psR", tag="pu")
            for qq in range(4):
                H = H2s.pop(4 * g + qq)
                H2 = h2_pool.tile([128, 1024], dt.bfloat16, name="H2", tag="H2")
                nc.vector.tensor_mul(H2[:], H[:], objT[:])
                for ph in range(2):
                    nc.tensor.matmul(
                        out=psR[0:4, :],
                        lhsT=OH[:, qq * 4 : (qq + 1) * 4],
                        rhs=H2[:, ph * 512 : (ph + 1) * 512],
                        start=(qq == 0 and ph == 0),
                        stop=(qq == 3 and ph == 1),
                    )
            outS = outs_pool.tile([4, B], dt.float32, name=f"oS{g}", tag="oS")
            nc.scalar.copy(out=outS[:], in_=psR[0:4, :])
            nc.sync.dma_start(out=out_p[4 * g : 4 * g + 4, :], in_=outS[:])

        # ================= program order =================
        gather("s", 0)
        gather("v", 0)
        gather("s", 1)
        gather("v", 1)
        gather("s", 2)
        gather("v", 2)
        gather("s", 3)
        gather("v", 3)
        load_piece(u_res, u_p, 0)
        load_piece(w_res, w_p, 0)
        gather("o", 0)
        gather("o", 1)
        gather("o", 2)
        gather("o", 3)
        load_piece(u_res, u_p, 1)
        load_piece(w_res, w_p, 1)
        load_piece(u_res, u_p, 2)
        load_piece(w_res, w_p, 2)
        load_piece(u_res, u_p, 3)
        load_piece(w_res, w_p, 3)

        for k in range(NQUART):
            bags("s", k)
            bags("v", k)
        for q in range(4):
            q_tile(q)
        # reduce_group(g) is emitted ~3 q_tiles after its last H so the
        # in-order PE queue never waits on DVE's H2 chain; o-bags are spread
        # through q4-q7 to track the o-gather arrivals.
        for q in range(4, QS):
            q_tile(q)
            if 4 <= q <= 7:
                bags("o", q - 4)
            if q == 8:
                reduce_group(0)
            elif q >= 10 and (q - 6) % 4 == 0:
                reduce_group((q - 6) // 4)
        reduce_group(7)

    nc.finalize()
    return nc


def _get_program():
    if "nc" not in _PROG_CACHE:
        _PROG_CACHE["nc"] = _build_program()
    return _PROG_CACHE["nc"]


def _host_prep(inputs):
    """Shard + lay out inputs for the 8 cores. Returns list of in_maps."""
    ids = {}
    wts = {}
    for t, idk, wk in (
        ("s", "subj_id", "subj_w"),
        ("v", "verb_id", "verb_w"),
        ("o", "obj_id", "obj_w"),
    ):
        ids[t] = np.asarray(inputs[idk]).astype(np.int32)
        wts[t] = np.asarray(inputs[wk]).astype(np.float32)

    emb = np.asarray(inputs["emb"], dtype=np.float32)
    w = np.asarray(inputs["w"], dtype=np.float32)
    u = np.asarray(inputs["u"], dtype=np.float32)

    emb_b = emb.astype(bf16)
    # [i, p, q] -> [i, q, p], contiguous, then shard q
    wT = np.ascontiguousarray(w.transpose(0, 2, 1)).astype(bf16)
    uT = np.ascontiguousarray(u.transpose(0, 2, 1)).astype(bf16)

    ids_r = {}
    S_cols = []
    for t in "svo":
        # partition p = (c % 16)*8 + n ; column = chunk ck = c // 16
        ids_r[t] = ids[t].reshape(NCHUNK, 16, 8).transpose(1, 2, 0).reshape(128, NCHUNK)
        Sm = np.zeros((16, 8, NCHUNK, 16), np.float32)
        wr = wts[t].reshape(NCHUNK, 16, 8).transpose(1, 2, 0)
        j = np.arange(16)
        Sm[j[:, None, None], np.arange(8)[None, :, None], np.arange(NCHUNK)[None, None, :], j[:, None, None]] = wr
        S_cols.append(Sm.reshape(128, B))
    S_all = np.ascontiguousarray(np.concatenate(S_cols, axis=1)).astype(bf16)

    # Compact the vocab to the unique used ids so indices fit int16 for
    # dma_gather; remap slice indices into the compacted table.
    uniq = np.unique(np.concatenate([ids_r[t].ravel() for t in "svo"]))
    emb_g = np.zeros((NG, D), dtype=bf16)
    emb_g[: len(uniq)] = emb_b[uniq]

    def slice_cols(t, c0, c1):
        # gathered row t_loc = (ck-c0)*128 + ppos -> idx tile [16, (c1-c0)*8]
        arr = ids_r[t][:, c0:c1]  # [128 ppos, nck]
        arr2 = arr.reshape(8, 16, c1 - c0).transpose(1, 2, 0).reshape(16, (c1 - c0) * 8)
        return np.searchsorted(uniq, arr2).astype(np.int16)

    idsg16 = np.concatenate(
        [slice_cols(t, k * 8, (k + 1) * 8) for t in "svo" for k in range(NQUART)],
        axis=1,
    )  # [16, 768]
    idsg = np.ascontiguousarray(np.tile(idsg16, (8, 1)))  # [128, 768]

    # one-hot blocks: OH[:, 4j:4j+4] has column j all ones, others zero
    oh = np.zeros((128, 16), np.float32)
    for jj in range(4):
        oh[:, 4 * jj + jj] = 1.0
    oh = oh.astype(bf16)

    in_maps = []
    for k in range(N_CORES):
        m = {
            "emb_g": emb_g,
            "w_k": np.ascontiguousarray(wT[:, k * QS : (k + 1) * QS, :]),
            "u_k": np.ascontiguousarray(uT[:, k * QS : (k + 1) * QS, :]),
            "idsg": idsg,
            "S": S_all,
            "oh": oh,
        }
        in_maps.append(m)
    return in_maps


def kernel(**inputs) -> np.ndarray:
    from concourse.bass_utils import run_bass_kernel_spmd

    nc = _get_program()
    in_maps = _host_prep(inputs)
    trace = bool(int(os.environ.get("KTRACE", "0")))
    res = run_bass_kernel_spmd(
        nc, in_maps, core_ids=list(range(N_CORES)), trace=trace
    )
    if trace:
        _PROG_CACHE["last_result"] = res
    # per-core outT is [QS, B]; transpose and concat along q
    out = np.concatenate(
        [res.results[k]["outT"].astype(np.float32).T for k in range(N_CORES)], axis=1
    )
    return out


# revision 52
# speedup vs baseline: 1.1779x; 1.0147x over previous
"""Trainium2 Bass kernel for nn_PredicateTensorModel.

Math (reference):
  subj/verb/obj[c,d] = weighted embedding bags (N=8 ids per batch row)
  A[c,p,q]  = sum_i w[i,p,q] verb[c,i]
  US[c,p,q] = sum_j u[j,p,q] subj[c,j]
  out[c,q]  = sum_p US[c,p,q] * A[c,p,q] * obj[c,p]

Sharding: tensor-parallel over trailing q axis (32 q's per core, 8 cores).

v3 design ([p,c] orientation):
  - Embedding rows are fetched with 12 dma_gather ops (1024 rows each --
    the SWDGE ring holds 1024 descriptors) from a host-compacted table
    (unique used ids remapped to int16 range), replacing the baseline's
    96 x ~1us single-row SWDGE gathers: ~16us Pool gen instead of ~100us.
  - Per q: psU'[p, c] = u_slice^T @ embT_s (lhsT = u[i, p-half], rhs =
    embT_s[i, c]); psA'[p, c] likewise; both [128, 1024] PSUM tiles
    (ph0|ph1 columns).  Act extracts psU'->USs (bf16), DVE computes
    H' = psA' * USs (one PSUM operand) and H'' = H' * objT (2x mode,
    all-SBUF bf16).  objT[p, c] is the o-bag in its natural [d, c]
    orientation - no transposes anywhere.
  - The p-reduction runs on the PE: out[q-slot, c] = onehot_q^T @ H''
    accumulated over 4 q x 2 ph into a [4, 512] PSUM tile, DMA'd
    directly to DRAM (host transposes [32, 512] -> [512, 32]).
  - Engine budget per core: PE ~73us (54.6 main + 13.7 reduce + bags),
    DVE ~63, Act ~47, Pool ~9, DMA ~43.
"""

import os
import sys

sys.path.insert(0, "/opt/trn_rl_repo")

import numpy as np
import ml_dtypes

N_CORES = 8
VOCAB, D, B, N = 50000, 256, 512, 8
QS = D // N_CORES  # 32 q columns per core
NCHUNK = B // 16  # 32 gather chunks of 16 batch rows
NQUART = 4
CHQ = NCHUNK // NQUART  # 8 chunks per quarter
NG = 3 * B * N  # total gathered rows (12288); compacted emb table size

bf16 = ml_dtypes.bfloat16

_PROG_CACHE = {}


def _build_program():
    import concourse.bass as bass
    import concourse.tile as tile
    import concourse.mybir as mybir
    from concourse import bacc
    from contextlib import ExitStack

    dt = mybir.dt
    # 2048-descriptor SWDGE ring: consecutive 1024-row dma_gathers
    # double-buffer through it instead of serializing gen->DMA->gen.
    nc = bacc.Bacc(dynamic_dma_scratch_size=32768)

    # compacted embedding table: rows = unique used ids (host-remapped)
    emb_p = nc.declare_dram_parameter("emb_g", [NG, D], dt.bfloat16, isOutput=False)
    w_p = nc.declare_dram_parameter("w_k", [D, QS, D], dt.bfloat16, isOutput=False)
    u_p = nc.declare_dram_parameter("u_k", [D, QS, D], dt.bfloat16, isOutput=False)
    # int16 dma_gather indices, 16-partition-wrapped per gather slice:
    # cols = [s ch0-8 (64) | s ch8-32 (192) | v0 (64) | v1 (192) | o (256)]
    ids_p = nc.declare_dram_parameter("idsg", [128, 768], dt.int16, isOutput=False)
    # S: columns = [S_s | S_v | S_o], each [128, B]
    S_p = nc.declare_dram_parameter("S", [128, 3 * B], dt.bfloat16, isOutput=False)
    # one-hot blocks for the PE reduce: block j (cols 4j..4j+3) = e_j outer ones
    oh_p = nc.declare_dram_parameter("oh", [128, 16], dt.bfloat16, isOutput=False)
    # output transposed: [q, c] per core; host transposes back
    out_p = nc.declare_dram_parameter("outT", [QS, B], dt.float32, isOutput=True)

    from concourse.library_config import mlp

    with ExitStack() as ctx:
        tc = ctx.enter_context(tile.TileContext(nc))
        nc.gpsimd.load_library(mlp)
        const_pool = ctx.enter_context(tc.tile_pool(name="const", bufs=1))
        gpool = ctx.enter_context(tc.tile_pool(name="gather", bufs=1))
        wu_pool = ctx.enter_context(tc.tile_pool(name="wu", bufs=1))
        embT_pool = ctx.enter_context(tc.tile_pool(name="embT", bufs=1))
        uss_pool = ctx.enter_context(tc.tile_pool(name="uss", bufs=4))
        h_pool = ctx.enter_context(tc.tile_pool(name="h", bufs=10))
        h2_pool = ctx.enter_context(tc.tile_pool(name="h2", bufs=5))
        outs_pool = ctx.enter_context(tc.tile_pool(name="outs", bufs=2))
        pu_pool = ctx.enter_context(tc.tile_pool(name="pu", bufs=2, space="PSUM"))
        pa_pool = ctx.enter_context(tc.tile_pool(name="pa", bufs=2, space="PSUM"))

        # ---- constants ----
        ids = const_pool.tile([128, 768], dt.int16, name="ids", tag="ids")
        nc.sync.dma_start(out=ids[:], in_=ids_p[:])
        S = const_pool.tile([128, 3 * B], dt.bfloat16, name="S", tag="S")
        OH = const_pool.tile([128, 16], dt.bfloat16, name="oh", tag="oh")
        nc.sync.dma_start(out=OH[:], in_=oh_p[:])

        # ---- resident w/u tiles: [128 i(half), 32q x 256p] per ic ----
        w_res = []
        u_res = []
        for ic in range(2):
            w_res.append(wu_pool.tile([128, QS * D], dt.bfloat16, name=f"w{ic}", tag=f"w{ic}"))
            u_res.append(wu_pool.tile([128, QS * D], dt.bfloat16, name=f"u{ic}", tag=f"u{ic}"))

        def load_piece(res, par, p):
            # one [128, 2048] piece = 8 q columns for one ic half.
            # SWDGE (Pool queue) so the DMA requests stay ordered *behind*
            # the gathers emitted before them in program order; HWDGE
            # requests would jump the queue and starve the gather stream.
            for ic in range(2):
                nc.gpsimd.dma_start(
                    out=res[ic][:, p * 2048 : (p + 1) * 2048],
                    in_=par[ic * 128 : (ic + 1) * 128, p * 8 : (p + 1) * 8, :],
                )

        # ---- gather tiles: one [128, 8*D] tile per (type, quarter) ----
        # each dma_gather moves 1024 rows (8 chunks) = 1024 descriptors,
        # fitting the 1024-descriptor SWDGE ring carveout.
        V = {
            t: [
                gpool.tile([128, 8 * D], dt.bfloat16, name=f"V{t}{k}", tag=f"V{t}{k}")
                for k in range(NQUART)
            ]
            for t in "svo"
        }
        # idsg column order: [s q0..q3 | v q0..q3 | o q0..q3], 64 cols each
        TOFF = {"s": 0, "v": 256, "o": 512}

        def gather(t, k):
            off = TOFF[t] + k * 64
            nc.gpsimd.dma_gather(
                out_ap=V[t][k][:].rearrange("p (k c) -> p k c", c=D),
                in_ap=emb_p[:],
                idxs_ap=ids[:, off : off + 64],
                num_idxs=1024,
                num_idxs_reg=1024,
                elem_size=D,
            )

        def vslice(t, ck):
            return V[t][ck // 8][:, (ck % 8) * D : (ck % 8 + 1) * D]

        # ---- bags: embT[t][dh] = [128 d-half, 512 c] bf16 ----
        embT = {
            t: [
                embT_pool.tile([128, B], dt.bfloat16, name=f"eT{t}{dh}", tag=f"eT{t}{dh}")
                for dh in range(2)
            ]
            for t in "sv"
        }
        # objT: [128 p(-half along cols), ph*512 + c] matching H' layout
        objT = embT_pool.tile([128, 2 * B], dt.bfloat16, name="objT", tag="objT")

        def bags(t, k):
            toff = {"s": 0, "v": B, "o": 2 * B}[t]
            psE = pu_pool.tile([128, 256], dt.float32, name=f"psE{t}{k}", tag="pu")
            for dh in range(2):
                for c8 in range(CHQ):
                    ck = k * CHQ + c8
                    nc.tensor.matmul(
                        out=psE[:, dh * 128 + c8 * 16 : dh * 128 + (c8 + 1) * 16],
                        lhsT=vslice(t, ck)[:, dh * 128 : (dh + 1) * 128],
                        rhs=S[:, toff + ck * 16 : toff + (ck + 1) * 16],
                        start=True,
                        stop=True,
                    )
            cb = k * 128
            for dh in range(2):
                if t == "o":
                    nc.scalar.copy(
                        out=objT[:, dh * 512 + cb : dh * 512 + cb + 128],
                        in_=psE[:, dh * 128 : (dh + 1) * 128],
                    )
                else:
                    nc.scalar.copy(
                        out=embT[t][dh][:, cb : cb + 128],
                        in_=psE[:, dh * 128 : (dh + 1) * 128],
                    )

        # ---- per-q pipeline ----
        H2s = {}

        def q_tile(q):
            # psU'[p, c]: cols = ph*512 + c
            psU = pu_pool.tile([128, 1024], dt.float32, name="psU", tag="pu")
            for ph in range(2):
                for ic in range(2):
                    nc.tensor.matmul(
                        out=psU[:, ph * 512 : (ph + 1) * 512],
                        lhsT=u_res[ic][:, q * 256 + ph * 128 : q * 256 + (ph + 1) * 128],
                        rhs=embT["s"][ic][:],
                        start=(ic == 0),
                        stop=(ic == 1),
                    )
            USs = uss_pool.tile([128, 1024], dt.bfloat16, name="USs", tag="USs")
            nc.scalar.copy(out=USs[:], in_=psU[:])
            psA = pa_pool.tile([128, 1024], dt.float32, name="psA", tag="pa")
            for ph in range(2):
                for ic in range(2):
                    nc.tensor.matmul(
                        out=psA[:, ph * 512 : (ph + 1) * 512],
                        lhsT=w_res[ic][:, q * 256 + ph * 128 : q * 256 + (ph + 1) * 128],
                        rhs=embT["v"][ic][:],
                        start=(ic == 0),
                        stop=(ic == 1),
                    )
            H = h_pool.tile([128, 1024], dt.bfloat16, name="H", tag="H")
            nc.vector.tensor_mul(H[:], psA[:], USs[:])
            H2s[q] = H

        def reduce_group(g):
            # H2 = H * objT (deferred here so it follows the o-bags), then
            # psR[0:4, c] = sum_{ph} onehot_qq^T @ H2(q=4g+qq)[ph] for qq in 0..3
            psR = pu_pool.tile([128, 512], dt.float32, name="psR", tag="pu")
            for qq in range(4):
                H = H2s.pop(4 * g + qq)
                H2 = h2_pool.tile([128, 1024], dt.bfloat16, name="H2", tag="H2")
                nc.vector.tensor_mul(H2[:], H[:], objT[:])
                for ph in range(2):
                    nc.tensor.matmul(
                        out=psR[0:4, :],
                        lhsT=OH[:, qq * 4 : (qq + 1) * 4],
                        rhs=H2[:, ph * 512 : (ph + 1) * 512],
                        start=(qq == 0 and ph == 0),
                        stop=(qq == 3 and ph == 1),
                    )
            outS = outs_pool.tile([4, B], dt.float32, name=f"oS{g}", tag="oS")
            nc.scalar.copy(out=outS[:], in_=psR[0:4, :])
            nc.sync.dma_start(out=out_p[4 * g : 4 * g + 4, :], in_=outS[:])

        # ================= program order =================
        gather("s", 0)
        gather("v", 0)
        # S rides the Pool queue behind the first two gathers: its DMA slot
        # no longer delays the first gather, yet it lands before the bags.
        nc.gpsimd.dma_start(out=S[:], in_=S_p[:])
        gather("s", 1)
        gather("v", 1)
        gather("s", 2)
        gather("v", 2)
        gather("s", 3)
        gather("v", 3)
        load_piece(u_res, u_p, 0)
        load_piece(w_res, w_p, 0)
        gather("o", 0)
        gather("o", 1)
        gather("o", 2)
        gather("o", 3)
        load_piece(u_res, u_p, 1)
        load_piece(w_res, w_p, 1)
        load_piece(u_res, u_p, 2)
        load_piece(w_res, w_p, 2)
        load_piece(u_res, u_p, 3)
        load_piece(w_res, w_p, 3)

        for k in range(NQUART):
            bags("s", k)
            bags("v", k)
        for q in range(4):
            q_tile(q)
        # reduce_group(g) is emitted ~3 q_tiles after its last H so the
        # in-order PE queue never waits on DVE's H2 chain; o-bags are spread
        # through q4-q7 to track the o-gather arrivals.
        for q in range(4, QS):
            q_tile(q)
            if 4 <= q <= 7:
                bags("o", q - 4)
            if q == 8:
                reduce_group(0)
            elif q >= 10 and (q - 6) % 4 == 0:
                reduce_group((q - 6) // 4)
        reduce_group(7)

    nc.finalize()
    return nc


def _get_program():
    if "nc" not in _PROG_CACHE:
        _PROG_CACHE["nc"] = _build_program()
    return _PROG_CACHE["nc"]


def _host_prep(inputs):
    """Shard + lay out inputs for the 8 cores. Returns list of in_maps."""
    ids = {}
    wts = {}
    for t, idk, wk in (
        ("s", "subj_id", "subj_w"),
        ("v", "verb_id", "verb_w"),
        ("o", "obj_id", "obj_w"),
    ):
        ids[t] = np.asarray(inputs[idk]).astype(np.int32)
        wts[t] = np.asarray(inputs[wk]).astype(np.float32)

    emb = np.asarray(inputs["emb"], dtype=np.float32)
    w = np.asarray(inputs["w"], dtype=np.float32)
    u = np.asarray(inputs["u"], dtype=np.float32)

    emb_b = emb.astype(bf16)
    # [i, p, q] -> [i, q, p], contiguous, then shard q
    wT = np.ascontiguousarray(w.transpose(0, 2, 1)).astype(bf16)
    uT = np.ascontiguousarray(u.transpose(0, 2, 1)).astype(bf16)

    ids_r = {}
    S_cols = []
    for t in "svo":
        # partition p = (c % 16)*8 + n ; column = chunk ck = c // 16
        ids_r[t] = ids[t].reshape(NCHUNK, 16, 8).transpose(1, 2, 0).reshape(128, NCHUNK)
        Sm = np.zeros((16, 8, NCHUNK, 16), np.float32)
        wr = wts[t].reshape(NCHUNK, 16, 8).transpose(1, 2, 0)
        j = np.arange(16)
        Sm[j[:, None, None], np.arange(8)[None, :, None], np.arange(NCHUNK)[None, None, :], j[:, None, None]] = wr
        S_cols.append(Sm.reshape(128, B))
    S_all = np.ascontiguousarray(np.concatenate(S_cols, axis=1)).astype(bf16)

    # Compact the vocab to the unique used ids so indices fit int16 for
    # dma_gather; remap slice indices into the compacted table.
    uniq = np.unique(np.concatenate([ids_r[t].ravel() for t in "svo"]))
    emb_g = np.zeros((NG, D), dtype=bf16)
    emb_g[: len(uniq)] = emb_b[uniq]

    def slice_cols(t, c0, c1):
        # gathered row t_loc = (ck-c0)*128 + ppos -> idx tile [16, (c1-c0)*8]
        arr = ids_r[t][:, c0:c1]  # [128 ppos, nck]
        arr2 = arr.reshape(8, 16, c1 - c0).transpose(1, 2, 0).reshape(16, (c1 - c0) * 8)
        return np.searchsorted(uniq, arr2).astype(np.int16)

    idsg16 = np.concatenate(
        [slice_cols(t, k * 8, (k + 1) * 8) for t in "svo" for k in range(NQUART)],
        axis=1,
    )  # [16, 768]
    idsg = np.ascontiguousarray(np.tile(idsg16, (8, 1)))  # [128, 768]

    # one-hot blocks: OH[:, 4j:4j+4] has column j all ones, others zero
    oh = np.zeros((128, 16), np.float32)
    for jj in range(4):
        oh[:, 4 * jj + jj] = 1.0
    oh = oh.astype(bf16)

    in_maps = []
    for k in range(N_CORES):
        m = {
            "emb_g": emb_g,
            "w_k": np.ascontiguousarray(wT[:, k * QS : (k + 1) * QS, :]),
            "u_k": np.ascontiguousarray(uT[:, k * QS : (k + 1) * QS, :]),
            "idsg": idsg,
            "S": S_all,
            "oh": oh,
        }
        in_maps.append(m)
    return in_maps


def kernel(**inputs) -> np.ndarray:
    from concourse.bass_utils import run_bass_kernel_spmd

    nc = _get_program()
    in_maps = _host_prep(inputs)
    trace = bool(int(os.environ.get("KTRACE", "0")))
    res = run_bass_kernel_spmd(
        nc, in_maps, core_ids=list(range(N_CORES)), trace=trace
    )
    if trace:
        _PROG_CACHE["last_result"] = res
    # per-core outT is [QS, B]; transpose and concat along q
    out = np.concatenate(
        [res.results[k]["outT"].astype(np.float32).T for k in range(N_CORES)], axis=1
    )
    return out
